# revision 1
# baseline (speedup 1.0000x reference)
"""DeBut-factorized 1D conv (kernel_size=4) on 8 Trainium2 NeuronCores.

Math: y[b,:,l] = W @ im2col_row(b,l) + bias, where W (512x2048) is a chain of
4 block-diagonal butterfly factors T4@T3@T2@T1. We fold T3@T2@T1 into stage A
(block-diagonal, 8 dense [128 out x 256 in] blocks) and keep T4 (diag-strided,
64 independent 8x16 matmuls) as stage B packed into [128x128] PE weight tiles
whose only nonzeros sit on 64-stride diagonals (the PE doesn't care).

Stage A im2col is free: the rhs for (window offset i, channel block) is just a
shifted free-dim slice of x[b] resident in SBUF.

Two kernel layouts (default mode "f16nat" = natural layout with fp16 inputs
and fp32 PSUM accumulation — fastest measured: ~41 us/exec steady state,
~5.8e-4 rel err):
  * natural ("f16nat"/"f32r"): all matmuls are full [K=128 -> M=128] at dst
    partition 0. 16 stage-A + 32 stage-B matmuls per 512-row tile; fp16 gets
    separate pulled-ahead LDWEIGHTS + 2-elem/cycle rhs streaming.
  * rotated ("f32"/"bf16"/"f16"): stage A split into M=32 column-tiles on the
    four PE column groups, arranged so stage B collapses to 8 matmuls.
    Measured slower on HW than the natural layout despite fewer matmuls.

Sharding: data-parallel over batch - each of the 8 cores takes 2 of the 16
batches; twiddle/bias-derived weights are replicated.
"""

import os
import numpy as np

# ---------------------------------------------------------------- constants
R_SHAPES = [[2048, 2048, 4, 4, 1],
            [2048, 2048, 8, 8, 4],
            [1024, 2048, 4, 8, 32],
            [512, 1024, 8, 16, 64]]
KW = 4              # conv kernel size
B_FULL, C_IN, L_IN = 16, 512, 2048
L_OUT = L_IN - (KW - 1)          # 2045
N_CORES = 8
B_SH = B_FULL // N_CORES         # 2 batches per core
NT = 512                         # l-tile width (PSUM bank = 512 f32)
# last tile overlaps by 3 columns (recomputed identically) so every tile is a
# uniform 512 wide — fp32r matmuls require an even free dim, and 512 keeps the
# window reads inside x's 2048 columns.
L_TILES = [(0, 512), (512, 512), (1024, 512), (1533, 512)]

# "f16nat": fp16 inputs, fp32 accumulate, natural layout (fastest measured:
#           separate LDWEIGHTS + FWL; ~5e-4 rel err)
# "f32r": reduced-precision full-rate fp32 PE path (natural layout, ~3e-4)
# "f32" : exact fp32, 4 cyc/row (rotated layout)
# "bf16"/"f16": 16-bit rotated col-tiled layouts (slower on HW)
MM_MODE = os.environ.get("DEBUT_MM_MODE", "f16nat")

_CACHE = {}          # mode -> compiled Bacc module
LAST_RESULT = None   # BassKernelResults of the most recent run (for test.py)


# ------------------------------------------------------- host-side weights
def _build_T_matrices(twiddle):
    Ts, p = [], 0
    for (osz, isz, row, col, diag) in R_SHAPES:
        g = isz // (col * diag)
        n_p = col * osz
        t = twiddle[p:p + n_p].reshape(g, diag, row, col).transpose(0, 2, 3, 1)
        T = np.zeros((osz, isz), dtype=np.float64)
        gi = np.arange(g)[:, None, None, None]
        ri = np.arange(row)[None, :, None, None]
        ci = np.arange(col)[None, None, :, None]
        di = np.arange(diag)[None, None, None, :]
        oi = np.broadcast_to((gi * row + ri) * diag + di, t.shape)
        ii = np.broadcast_to((gi * col + ci) * diag + di, t.shape)
        T[oi.ravel(), ii.ravel()] = t.ravel()
        Ts.append(T)
        p += n_p
    return Ts


def _make_weights(twiddle, bias, rotated):
    """Returns wA [16,128,128], wB list, bias_t [128,4] (f32).

    natural: wB has 32 tiles (h*8 + g3), y-tile h = features [128h, 128h+128).
    rotated: wB has 8 tiles (Dp*2 + P), y-tile Dp features r*64 + 16*Dp + j
             at partition m = r*16 + j.
    """
    T1, T2, T3, T4 = _build_T_matrices(twiddle.astype(np.float64))
    M321 = T3 @ T2 @ T1                       # block-diag: 8 x [128 out, 256 in]

    m = np.arange(128)
    if rotated:
        # stage-A M column m = Dp*32 + j*2 + b -> block-local feature b*64+16Dp+j
        Dp_of_m, s = m // 32, m % 32
        o_of_m = (s % 2) * 64 + 16 * Dp_of_m + (s // 2)
    else:
        o_of_m = m                            # natural order

    wA = np.zeros((16, 128, 128), dtype=np.float32)
    for g3 in range(8):
        blk = M321[g3 * 128:(g3 + 1) * 128, g3 * 256:(g3 + 1) * 256]
        for k in range(2):
            wA[g3 * 2 + k] = blk[o_of_m, k * 128:(k + 1) * 128].T.astype(np.float32)

    bias_t = np.zeros((128, 4), dtype=np.float32)
    if rotated:
        # tile (Dp, P): p = qp*32 + j*2 + b, logical q = (qp - Dp) % 4
        # (col-group rotation so concurrent col-tiles use distinct groups);
        # u feature c4*64 + d4 with c4 = 2*(4P+q)+b, d4 = 16*Dp+j.
        wB = np.zeros((8, 128, 128), dtype=np.float32)
        p_idx = np.arange(128)
        qp, sp = p_idx // 32, p_idx % 32
        jj, bb = sp // 2, sp % 2
        for Dp in range(4):
            q = (qp - Dp) % 4
            d4 = 16 * Dp + jj
            for P in range(2):
                c4 = 2 * (4 * P + q) + bb
                for r in range(8):
                    wB[Dp * 2 + P, p_idx, r * 16 + jj] = T4[r * 64 + d4, c4 * 64 + d4]
        r_of_m, j_of_m = m // 16, m % 16
        for Dp in range(4):
            bias_t[:, Dp] = bias[r_of_m * 64 + 16 * Dp + j_of_m]
    else:
        # tile (h, g3): p = b*64 + d4 (u-tile g3 local feature),
        # m = a*64 + d4' (y features 128h + m); nonzero iff d4' == d4:
        # wB[p, m] = T4[(2h+a)*64 + d4, (2*g3+b)*64 + d4]
        wB = np.zeros((32, 128, 128), dtype=np.float32)
        d4 = np.arange(64)
        for h in range(4):
            for g3 in range(8):
                for a in range(2):
                    for b in range(2):
                        wB[h * 8 + g3, b * 64 + d4, a * 64 + d4] = \
                            T4[(2 * h + a) * 64 + d4, (2 * g3 + b) * 64 + d4]
        for h in range(4):
            bias_t[:, h] = bias[128 * h + m]
    return wA, wB, bias_t


# ------------------------------------------------------------- bass kernel
def _emit(tc, nc, mybir, x, wA, wB, bt, y, mode, repeats=1):
    import contextlib
    f32 = mybir.dt.float32
    in_dt = {"f32r": mybir.dt.float32r, "f32": f32, "bf16": mybir.dt.bfloat16,
             "f16": mybir.dt.float16, "f16nat": mybir.dt.float16}[mode]
    rotated = mode in ("f32", "bf16", "f16")
    nB = 8 if rotated else 32
    # gpsimd DMA casts f32 -> 16-bit on the fly
    x_dma = nc.sync if mode in ("f32", "f32r") else nc.gpsimd

    ctx = contextlib.ExitStack()
    with ctx:
        wpool = ctx.enter_context(tc.tile_pool(name="wpool", bufs=1))
        xpool = ctx.enter_context(tc.tile_pool(name="xpool", bufs=12))
        upsum = ctx.enter_context(tc.tile_pool(name="upsum", bufs=4, space="PSUM"))
        ypsum = ctx.enter_context(tc.tile_pool(name="ypsum", bufs=4, space="PSUM"))
        usb = ctx.enter_context(tc.tile_pool(name="usb", bufs=24))
        ysb = ctx.enter_context(tc.tile_pool(name="ysb", bufs=12))

        wA_sb = []
        for i in range(16):
            t = wpool.tile([128, 128], in_dt, tag=f"wA{i}")
            nc.sync.dma_start(t[:], wA[i])
            wA_sb.append(t)
        wB_sb = []
        for i in range(nB):
            t = wpool.tile([128, 128], in_dt, tag=f"wB{i}")
            nc.sync.dma_start(t[:], wB[i])
            wB_sb.append(t)
        bt_sb = wpool.tile([128, 4], f32, tag="bt")
        nc.sync.dma_start(bt_sb[:], bt[:])

        if rotated:
            # y viewed as [b, r, Dp, j, l]: feature r*64 + 16*Dp + j; DMA pairs
            # the (r, j) dims against the 128 SBUF partitions (m = r*16 + j).
            yv = y.rearrange("b (r p j) l -> b r p j l", r=8, p=4, j=16)

        def stage_A_rot(xs, l0, nt):
            u_sbuf = {}
            for P in range(2):
                u_ps = [upsum.tile([128, NT], f32, tag="u", name=f"ups{P}_{d}")
                        for d in range(4)]
                for qq in range(4):
                    g3 = 4 * P + qq
                    i_off = g3 // 2
                    for k in range(2):
                        rhs = xs[(g3 % 2) * 2 + k][:, i_off:i_off + nt]
                        for Dp in range(4):
                            qp = (qq + Dp) % 4
                            nc.tensor.matmul(
                                u_ps[Dp][qp * 32:(qp + 1) * 32, :nt],
                                wA_sb[g3 * 2 + k][:, Dp * 32:(Dp + 1) * 32],
                                rhs,
                                start=(k == 0), stop=(k == 1),
                                tile_position=(0, qp * 32),
                            )
                for Dp in range(4):
                    t = usb.tile([128, NT], in_dt, tag="u_sb")
                    nc.vector.tensor_copy(t[:, :nt], u_ps[Dp][:, :nt])
                    u_sbuf[(Dp, P)] = t
            return u_sbuf

        def stage_B_rot(u_sbuf, b, l0, nt):
            for Dp in range(4):
                y_ps = ypsum.tile([128, NT], f32, tag="y")
                for P in range(2):
                    nc.tensor.matmul(
                        y_ps[:, :nt],
                        wB_sb[Dp * 2 + P][:],
                        u_sbuf[(Dp, P)][:, :nt],
                        start=(P == 0), stop=(P == 1),
                    )
                t = ysb.tile([128, NT], f32, tag="y_sb")
                nc.scalar.activation(
                    t[:, :nt], y_ps[:, :nt],
                    mybir.ActivationFunctionType.Identity,
                    bias=bt_sb[:, Dp:Dp + 1],
                )
                nc.sync.dma_start(yv[b, :, Dp, :, l0:l0 + nt], t[:, :nt])

        def stage_A_nat(xs, l0, nt):
            u_sbuf = {}
            for half in range(2):
                u_ps = [upsum.tile([128, NT], f32, tag="u", name=f"ups{half}_{d}")
                        for d in range(4)]
                for gg in range(4):
                    g3 = 4 * half + gg
                    i_off = g3 // 2
                    for k in range(2):
                        rhs = xs[(g3 % 2) * 2 + k][:, i_off:i_off + nt]
                        nc.tensor.matmul(
                            u_ps[gg][:, :nt],
                            wA_sb[g3 * 2 + k][:],
                            rhs,
                            start=(k == 0), stop=(k == 1),
                        )
                for gg in range(4):
                    g3 = 4 * half + gg
                    t = usb.tile([128, NT], in_dt, tag="u_sb")
                    nc.vector.tensor_copy(t[:, :nt], u_ps[gg][:, :nt])
                    u_sbuf[g3] = t
            return u_sbuf

        def stage_B_nat(u_sbuf, b, l0, nt):
            for h in range(4):
                y_ps = ypsum.tile([128, NT], f32, tag="y")
                for g3 in range(8):
                    nc.tensor.matmul(
                        y_ps[:, :nt],
                        wB_sb[h * 8 + g3][:],
                        u_sbuf[g3][:, :nt],
                        start=(g3 == 0), stop=(g3 == 7),
                    )
                t = ysb.tile([128, NT], f32, tag="y_sb")
                nc.scalar.activation(
                    t[:, :nt], y_ps[:, :nt],
                    mybir.ActivationFunctionType.Identity,
                    bias=bt_sb[:, h:h + 1],
                )
                nc.sync.dma_start(y[b, h * 128:(h + 1) * 128, l0:l0 + nt],
                                  t[:, :nt])

        stage_A = stage_A_rot if rotated else stage_A_nat
        stage_B = stage_B_rot if rotated else stage_B_nat

        # software-pipelined emission: stage B of iteration t is emitted after
        # stage A of iteration t+1 so the PE never waits on u copies.
        # repeats > 1 re-emits the whole body (benchmarking only).
        pending = None
        for _rep in range(repeats):
            for b in range(B_SH):
                for (l0, nt) in L_TILES:
                    xs = []
                    for t4i in range(4):
                        # per-l-tile window [128, nt+3]: first matmuls start
                        # after ~260KB instead of a full 4MB batch load
                        xt = xpool.tile([128, NT + KW - 1], in_dt, tag="x")
                        x_dma.dma_start(
                            xt[:, :nt + KW - 1],
                            x[b, t4i * 128:(t4i + 1) * 128, l0:l0 + nt + KW - 1])
                        xs.append(xt)
                    u_sbuf = stage_A(xs, l0, nt)
                    if pending is not None:
                        stage_B(*pending)
                    pending = (u_sbuf, b, l0, nt)
        stage_B(*pending)


def _get_module(mode, repeats=1):
    key = (mode, repeats)
    if key in _CACHE:
        return _CACHE[key]
    import concourse.mybir as mybir
    import concourse.tile as tile
    from concourse import bacc

    nc = bacc.Bacc("TRN2", target_bir_lowering=False, debug=False,
                   enable_asserts=False, num_devices=N_CORES)
    f32 = mybir.dt.float32
    w_dt = {"f32r": mybir.dt.float32r, "f32": f32, "bf16": mybir.dt.bfloat16,
            "f16": mybir.dt.float16, "f16nat": mybir.dt.float16}[mode]
    x_dt = f32 if mode in ("bf16", "f16", "f16nat") else w_dt
    nB = 8 if mode in ("f32", "bf16", "f16") else 32
    x = nc.dram_tensor("x", [B_SH, C_IN, L_IN], x_dt, kind="ExternalInput").ap()
    wA = nc.dram_tensor("wA", [16, 128, 128], w_dt, kind="ExternalInput").ap()
    wB = nc.dram_tensor("wB", [nB, 128, 128], w_dt, kind="ExternalInput").ap()
    bt = nc.dram_tensor("bt", [128, 4], f32, kind="ExternalInput").ap()
    y = nc.dram_tensor("y", [B_SH, 512, L_OUT], f32, kind="ExternalOutput").ap()

    with tile.TileContext(nc) as tc:
        _emit(tc, nc, mybir, x, wA, wB, bt, y, mode, repeats)
    nc.compile()
    _CACHE[key] = nc
    return nc


# ------------------------------------------------------------ entry point
def kernel(x, twiddle, bias):
    global LAST_RESULT
    from concourse import bass_utils

    x = np.ascontiguousarray(np.asarray(x), dtype=np.float32)
    twiddle = np.asarray(twiddle, dtype=np.float32)
    bias = np.asarray(bias, dtype=np.float32)

    wA, wB, bt = _make_weights(twiddle, bias,
                               rotated=(MM_MODE in ("f32", "bf16", "f16")))
    if MM_MODE == "bf16":
        import ml_dtypes
        wA = wA.astype(ml_dtypes.bfloat16)
        wB = wB.astype(ml_dtypes.bfloat16)
    elif MM_MODE in ("f16", "f16nat"):
        wA = wA.astype(np.float16)
        wB = wB.astype(np.float16)
    nc = _get_module(MM_MODE)

    in_maps = [
        {"x": x[c * B_SH:(c + 1) * B_SH], "wA": wA, "wB": wB, "bt": bt}
        for c in range(N_CORES)
    ]
    res = bass_utils.run_bass_kernel_spmd(nc, in_maps, list(range(N_CORES)))
    LAST_RESULT = res
    out = np.concatenate([res.results[c]["y"] for c in range(N_CORES)], axis=0)
    return out.astype(np.float32)



# revision 10
# speedup vs baseline: 5070.1440x; 5070.1440x over previous
"""DeBut-factorized 1D conv (kernel_size=4) on 8 Trainium2 NeuronCores.

Math: y[b,:,l] = W @ im2col_row(b,l) + bias, where W (512x2048) is a chain of
4 block-diagonal butterfly factors T4@T3@T2@T1. We fold T3@T2@T1 into stage A
(block-diagonal, 8 dense [128 out x 256 in] blocks) and keep T4 (diag-strided,
64 independent 8x16 matmuls) as stage B packed into [128x128] PE weight tiles
whose only nonzeros sit on 64-stride diagonals (the PE doesn't care).

Stage A im2col is free: the rhs for (window offset i, channel block) is just a
shifted free-dim slice of x[b] resident in SBUF.

Two kernel layouts (default mode "f16nat" = natural layout with fp16 inputs
and fp32 PSUM accumulation — fastest measured: ~41 us/exec steady state,
~5.8e-4 rel err):
  * natural ("f16nat"/"f32r"): all matmuls are full [K=128 -> M=128] at dst
    partition 0. 16 stage-A + 32 stage-B matmuls per 512-row tile; fp16 gets
    separate pulled-ahead LDWEIGHTS + 2-elem/cycle rhs streaming.
  * rotated ("f32"/"bf16"/"f16"): stage A split into M=32 column-tiles on the
    four PE column groups, arranged so stage B collapses to 8 matmuls.
    Measured slower on HW than the natural layout despite fewer matmuls.

Sharding: data-parallel over batch - each of the 8 cores takes 2 of the 16
batches; twiddle/bias-derived weights are replicated.
"""

import os
import numpy as np

# ---------------------------------------------------------------- constants
R_SHAPES = [[2048, 2048, 4, 4, 1],
            [2048, 2048, 8, 8, 4],
            [1024, 2048, 4, 8, 32],
            [512, 1024, 8, 16, 64]]
KW = 4              # conv kernel size
B_FULL, C_IN, L_IN = 16, 512, 2048
L_OUT = L_IN - (KW - 1)          # 2045
N_CORES = 8
B_SH = B_FULL // N_CORES         # 2 batches per core
NT = 512                         # l-tile width (PSUM bank = 512 f32)
# last tile overlaps by 3 columns (recomputed identically) so every tile is a
# uniform 512 wide — fp32r matmuls require an even free dim, and 512 keeps the
# window reads inside x's 2048 columns.
L_TILES = [(0, 512), (512, 512), (1024, 512), (1533, 512)]

# "f16io": like f16nat but x is pre-cast to fp16 on the host (halves x HBM
#          traffic, frees the gpsimd cast) and y is stored as fp16 on device
#          (halves y HBM traffic); u copies split across DVE and ACT.
# "f16nat": fp16 inputs, fp32 accumulate, natural layout (fastest measured:
#           separate LDWEIGHTS + FWL; ~5e-4 rel err)
# "f32r": reduced-precision full-rate fp32 PE path (natural layout, ~3e-4)
# "f32" : exact fp32, 4 cyc/row (rotated layout)
# "bf16"/"f16": 16-bit rotated col-tiled layouts (slower on HW)
MM_MODE = os.environ.get("DEBUT_MM_MODE", "f16io")

_CACHE = {}          # mode -> compiled Bacc module
LAST_RESULT = None   # BassKernelResults of the most recent run (for test.py)


# ------------------------------------------------------- host-side weights
def _build_T_matrices(twiddle):
    Ts, p = [], 0
    for (osz, isz, row, col, diag) in R_SHAPES:
        g = isz // (col * diag)
        n_p = col * osz
        t = twiddle[p:p + n_p].reshape(g, diag, row, col).transpose(0, 2, 3, 1)
        T = np.zeros((osz, isz), dtype=np.float64)
        gi = np.arange(g)[:, None, None, None]
        ri = np.arange(row)[None, :, None, None]
        ci = np.arange(col)[None, None, :, None]
        di = np.arange(diag)[None, None, None, :]
        oi = np.broadcast_to((gi * row + ri) * diag + di, t.shape)
        ii = np.broadcast_to((gi * col + ci) * diag + di, t.shape)
        T[oi.ravel(), ii.ravel()] = t.ravel()
        Ts.append(T)
        p += n_p
    return Ts


def _make_weights(twiddle, bias, rotated):
    """Returns wA [16,128,128], wB list, bias_t [128,4] (f32).

    natural: wB has 32 tiles (h*8 + g3), y-tile h = features [128h, 128h+128).
    rotated: wB has 8 tiles (Dp*2 + P), y-tile Dp features r*64 + 16*Dp + j
             at partition m = r*16 + j.
    """
    T1, T2, T3, T4 = _build_T_matrices(twiddle.astype(np.float64))
    M321 = T3 @ T2 @ T1                       # block-diag: 8 x [128 out, 256 in]

    m = np.arange(128)
    if rotated:
        # stage-A M column m = Dp*32 + j*2 + b -> block-local feature b*64+16Dp+j
        Dp_of_m, s = m // 32, m % 32
        o_of_m = (s % 2) * 64 + 16 * Dp_of_m + (s // 2)
    else:
        o_of_m = m                            # natural order

    wA = np.zeros((16, 128, 128), dtype=np.float32)
    for g3 in range(8):
        blk = M321[g3 * 128:(g3 + 1) * 128, g3 * 256:(g3 + 1) * 256]
        for k in range(2):
            wA[g3 * 2 + k] = blk[o_of_m, k * 128:(k + 1) * 128].T.astype(np.float32)

    bias_t = np.zeros((128, 4), dtype=np.float32)
    if rotated:
        # tile (Dp, P): p = qp*32 + j*2 + b, logical q = (qp - Dp) % 4
        # (col-group rotation so concurrent col-tiles use distinct groups);
        # u feature c4*64 + d4 with c4 = 2*(4P+q)+b, d4 = 16*Dp+j.
        wB = np.zeros((8, 128, 128), dtype=np.float32)
        p_idx = np.arange(128)
        qp, sp = p_idx // 32, p_idx % 32
        jj, bb = sp // 2, sp % 2
        for Dp in range(4):
            q = (qp - Dp) % 4
            d4 = 16 * Dp + jj
            for P in range(2):
                c4 = 2 * (4 * P + q) + bb
                for r in range(8):
                    wB[Dp * 2 + P, p_idx, r * 16 + jj] = T4[r * 64 + d4, c4 * 64 + d4]
        r_of_m, j_of_m = m // 16, m % 16
        for Dp in range(4):
            bias_t[:, Dp] = bias[r_of_m * 64 + 16 * Dp + j_of_m]
    else:
        # tile (h, g3): p = b*64 + d4 (u-tile g3 local feature),
        # m = a*64 + d4' (y features 128h + m); nonzero iff d4' == d4:
        # wB[p, m] = T4[(2h+a)*64 + d4, (2*g3+b)*64 + d4]
        wB = np.zeros((32, 128, 128), dtype=np.float32)
        d4 = np.arange(64)
        for h in range(4):
            for g3 in range(8):
                for a in range(2):
                    for b in range(2):
                        wB[h * 8 + g3, b * 64 + d4, a * 64 + d4] = \
                            T4[(2 * h + a) * 64 + d4, (2 * g3 + b) * 64 + d4]
        for h in range(4):
            bias_t[:, h] = bias[128 * h + m]
    return wA, wB, bias_t


# ------------------------------------------------------------- bass kernel
def _emit(tc, nc, mybir, x, wA, wB, bt, y, mode, repeats=1):
    import contextlib
    f32 = mybir.dt.float32
    in_dt = {"f32r": mybir.dt.float32r, "f32": f32, "bf16": mybir.dt.bfloat16,
             "f16": mybir.dt.float16, "f16nat": mybir.dt.float16,
             "f16io": mybir.dt.float16}[mode]
    rotated = mode in ("f32", "bf16", "f16")
    nB = 8 if rotated else 32
    y_dt = f32 if mode != "f16io" else mybir.dt.float16
    # gpsimd DMA casts f32 -> 16-bit on the fly (needed by the 16-bit modes
    # whose x arrives as f32). f16io x is host-cast, so it rides the sync
    # HWDGE queue — SWDGE descriptor writes can stall behind DVE SBUF-port
    # locks while DVE streams u copies.
    x_dma = nc.sync if mode in ("f32", "f32r", "f16io") else nc.gpsimd

    ctx = contextlib.ExitStack()
    with ctx:
        wpool = ctx.enter_context(tc.tile_pool(name="wpool", bufs=1))
        xpool = ctx.enter_context(
            tc.tile_pool(name="xpool",
                         bufs=8 if mode in ("f16nat", "f16io") else 12))
        upsum = ctx.enter_context(tc.tile_pool(name="upsum", bufs=4, space="PSUM"))
        ypsum = ctx.enter_context(tc.tile_pool(name="ypsum", bufs=4, space="PSUM"))
        usb = ctx.enter_context(tc.tile_pool(name="usb", bufs=24))
        ysb = ctx.enter_context(tc.tile_pool(name="ysb", bufs=12))

        wA_sb = []
        for i in range(16):
            t = wpool.tile([128, 128], in_dt, tag=f"wA{i}")
            nc.sync.dma_start(t[:], wA[i])
            wA_sb.append(t)
        wB_sb = []
        for i in range(nB):
            t = wpool.tile([128, 128], in_dt, tag=f"wB{i}")
            nc.sync.dma_start(t[:], wB[i])
            wB_sb.append(t)
        bt_sb = wpool.tile([128, 4], f32, tag="bt")
        nc.sync.dma_start(bt_sb[:], bt[:])

        if rotated:
            # y viewed as [b, r, Dp, j, l]: feature r*64 + 16*Dp + j; DMA pairs
            # the (r, j) dims against the 128 SBUF partitions (m = r*16 + j).
            yv = y.rearrange("b (r p j) l -> b r p j l", r=8, p=4, j=16)

        def stage_A_rot(xs, l0, nt):
            u_sbuf = {}
            for P in range(2):
                u_ps = [upsum.tile([128, NT], f32, tag="u", name=f"ups{P}_{d}")
                        for d in range(4)]
                for qq in range(4):
                    g3 = 4 * P + qq
                    i_off = g3 // 2
                    for k in range(2):
                        rhs = xs[(g3 % 2) * 2 + k][:, i_off:i_off + nt]
                        for Dp in range(4):
                            qp = (qq + Dp) % 4
                            nc.tensor.matmul(
                                u_ps[Dp][qp * 32:(qp + 1) * 32, :nt],
                                wA_sb[g3 * 2 + k][:, Dp * 32:(Dp + 1) * 32],
                                rhs,
                                start=(k == 0), stop=(k == 1),
                                tile_position=(0, qp * 32),
                            )
                for Dp in range(4):
                    t = usb.tile([128, NT], in_dt, tag="u_sb")
                    nc.vector.tensor_copy(t[:, :nt], u_ps[Dp][:, :nt])
                    u_sbuf[(Dp, P)] = t
            return u_sbuf

        def stage_B_rot(u_sbuf, b, l0, nt):
            for Dp in range(4):
                y_ps = ypsum.tile([128, NT], f32, tag="y")
                for P in range(2):
                    nc.tensor.matmul(
                        y_ps[:, :nt],
                        wB_sb[Dp * 2 + P][:],
                        u_sbuf[(Dp, P)][:, :nt],
                        start=(P == 0), stop=(P == 1),
                    )
                t = ysb.tile([128, NT], f32, tag="y_sb")
                nc.scalar.activation(
                    t[:, :nt], y_ps[:, :nt],
                    mybir.ActivationFunctionType.Identity,
                    bias=bt_sb[:, Dp:Dp + 1],
                )
                nc.sync.dma_start(yv[b, :, Dp, :, l0:l0 + nt], t[:, :nt])

        def stage_A_nat(xs, l0, nt):
            u_sbuf = {}
            for half in range(2):
                u_ps = [upsum.tile([128, NT], f32, tag="u", name=f"ups{half}_{d}")
                        for d in range(4)]
                for gg in range(4):
                    g3 = 4 * half + gg
                    i_off = g3 // 2
                    for k in range(2):
                        rhs = xs[(g3 % 2) * 2 + k][:, i_off:i_off + nt]
                        nc.tensor.matmul(
                            u_ps[gg][:, :nt],
                            wA_sb[g3 * 2 + k][:],
                            rhs,
                            start=(k == 0), stop=(k == 1),
                        )
                for gg in range(4):
                    g3 = 4 * half + gg
                    t = usb.tile([128, NT], in_dt, tag="u_sb")
                    # f16io: route one of each 4 PSUM->SBUF u copies to the
                    # scalar engine so DVE isn't the copy bottleneck.
                    if mode == "f16io" and gg == 3:
                        nc.scalar.copy(t[:, :nt], u_ps[gg][:, :nt])
                    else:
                        nc.vector.tensor_copy(t[:, :nt], u_ps[gg][:, :nt])
                    u_sbuf[g3] = t
            return u_sbuf

        def stage_B_nat(u_sbuf, b, l0, nt):
            for h in range(4):
                y_ps = ypsum.tile([128, NT], f32, tag="y")
                for g3 in range(8):
                    nc.tensor.matmul(
                        y_ps[:, :nt],
                        wB_sb[h * 8 + g3][:],
                        u_sbuf[g3][:, :nt],
                        start=(g3 == 0), stop=(g3 == 7),
                    )
                t = ysb.tile([128, NT], y_dt, tag="y_sb")
                nc.scalar.activation(
                    t[:, :nt], y_ps[:, :nt],
                    mybir.ActivationFunctionType.Identity,
                    bias=bt_sb[:, h:h + 1],
                )
                nc.sync.dma_start(y[b, h * 128:(h + 1) * 128, l0:l0 + nt],
                                  t[:, :nt])

        stage_A = stage_A_rot if rotated else stage_A_nat
        stage_B = stage_B_rot if rotated else stage_B_nat

        # software-pipelined emission: stage B of iteration t is emitted after
        # stage A of iteration t+1 so the PE never waits on u copies.
        # repeats > 1 re-emits the whole body (benchmarking only).
        whole_batch_x = mode in ("f16nat", "f16io")
        if whole_batch_x:
            # whole-batch x tiles ([128, 2048] per channel tile: 4KB/partition
            # DMA lines, all l-tiles slice SBUF-resident x), with the DMAs
            # emitted one batch AHEAD so they never queue behind the previous
            # batch's y stores.
            loads = [b for _ in range(repeats) for b in range(B_SH)]
            xs_cache = {}

            def load_batch(i):
                b = loads[i]
                xs_cache[i] = []
                for t4i in range(4):
                    xt = xpool.tile([128, L_IN], in_dt, tag="x")
                    x_dma.dma_start(xt[:], x[b, t4i * 128:(t4i + 1) * 128, :])
                    xs_cache[i].append(xt)

            load_batch(0)
            pending = None
            for i, b in enumerate(loads):
                if i + 1 < len(loads):
                    load_batch(i + 1)
                xs_b = xs_cache.pop(i)
                for (l0, nt) in L_TILES:
                    xs = [t[:, l0:l0 + nt + KW - 1] for t in xs_b]
                    u_sbuf = stage_A(xs, l0, nt)
                    if pending is not None:
                        stage_B(*pending)
                    pending = (u_sbuf, b, l0, nt)
            stage_B(*pending)
        else:
            pending = None
            for _rep in range(repeats):
                for b in range(B_SH):
                    for (l0, nt) in L_TILES:
                        xs = []
                        for t4i in range(4):
                            # per-l-tile window [128, nt+3]: first matmuls
                            # start after ~260KB instead of a full batch load
                            xt = xpool.tile([128, NT + KW - 1], in_dt, tag="x")
                            x_dma.dma_start(
                                xt[:, :nt + KW - 1],
                                x[b, t4i * 128:(t4i + 1) * 128,
                                  l0:l0 + nt + KW - 1])
                            xs.append(xt)
                        u_sbuf = stage_A(xs, l0, nt)
                        if pending is not None:
                            stage_B(*pending)
                        pending = (u_sbuf, b, l0, nt)
            stage_B(*pending)


def _get_module(mode, repeats=1):
    key = (mode, repeats)
    if key in _CACHE:
        return _CACHE[key]
    import concourse.mybir as mybir
    import concourse.tile as tile
    from concourse import bacc

    nc = bacc.Bacc("TRN2", target_bir_lowering=False, debug=False,
                   enable_asserts=False, num_devices=N_CORES)
    f32 = mybir.dt.float32
    w_dt = {"f32r": mybir.dt.float32r, "f32": f32, "bf16": mybir.dt.bfloat16,
            "f16": mybir.dt.float16, "f16nat": mybir.dt.float16,
            "f16io": mybir.dt.float16}[mode]
    x_dt = f32 if mode in ("bf16", "f16", "f16nat") else w_dt
    y_dt = mybir.dt.float16 if mode == "f16io" else f32
    nB = 8 if mode in ("f32", "bf16", "f16") else 32
    x = nc.dram_tensor("x", [B_SH, C_IN, L_IN], x_dt, kind="ExternalInput").ap()
    wA = nc.dram_tensor("wA", [16, 128, 128], w_dt, kind="ExternalInput").ap()
    wB = nc.dram_tensor("wB", [nB, 128, 128], w_dt, kind="ExternalInput").ap()
    bt = nc.dram_tensor("bt", [128, 4], f32, kind="ExternalInput").ap()
    y = nc.dram_tensor("y", [B_SH, 512, L_OUT], y_dt, kind="ExternalOutput").ap()

    with tile.TileContext(nc) as tc:
        _emit(tc, nc, mybir, x, wA, wB, bt, y, mode, repeats)
    nc.compile()
    _CACHE[key] = nc
    return nc


# ------------------------------------------------------------ entry point
def prepare_in_maps(x, twiddle, bias, mode=None):
    """Host-side prep shared by kernel() and the bench: weights from twiddle,
    per-core x shards (cast to fp16 for f16io)."""
    mode = mode or MM_MODE
    x = np.ascontiguousarray(np.asarray(x), dtype=np.float32)
    twiddle = np.asarray(twiddle, dtype=np.float32)
    bias = np.asarray(bias, dtype=np.float32)

    wA, wB, bt = _make_weights(twiddle, bias,
                               rotated=(mode in ("f32", "bf16", "f16")))
    if mode == "bf16":
        import ml_dtypes
        wA = wA.astype(ml_dtypes.bfloat16)
        wB = wB.astype(ml_dtypes.bfloat16)
    elif mode in ("f16", "f16nat", "f16io"):
        wA = wA.astype(np.float16)
        wB = wB.astype(np.float16)
    if mode == "f16io":
        x = x.astype(np.float16)
    return [
        {"x": x[c * B_SH:(c + 1) * B_SH], "wA": wA, "wB": wB, "bt": bt}
        for c in range(N_CORES)
    ]


def kernel(x, twiddle, bias):
    global LAST_RESULT
    from concourse import bass_utils

    in_maps = prepare_in_maps(x, twiddle, bias)
    nc = _get_module(MM_MODE)
    res = bass_utils.run_bass_kernel_spmd(nc, in_maps, list(range(N_CORES)))
    LAST_RESULT = res
    out = np.concatenate([res.results[c]["y"] for c in range(N_CORES)], axis=0)
    return out.astype(np.float32)

